# revision 2
# baseline (speedup 1.0000x reference)
"""GCNII (8-layer GCN2Conv stack) on 8 Trainium2 NeuronCores.

Strategy (1D graph parallel over destination nodes):
  - Core c owns destination rows [c*NS, (c+1)*NS) of every layer's output.
  - Edges partitioned by dst, sorted by dst, grouped per 128-dst-node "block"
    into 128-edge chunks (lo/hi split at LO_N so gather indices fit int16),
    padded with zero-weight edges to a cross-core-uniform chunk schedule.
  - Gathers are MERGED: consecutive blocks form groups of <=CAP chunks per
    src-half; one dma_gather per (group, half) fetches all rows (the Q7 SWDGE
    fixed cost ~7us/instruction dominated the old per-(block,half) scheme).
  - Selectors S[e, j] = (j == dstloc[e]) * w[e] are built per block with TWO
    broadcast tensor_tensor ops (iota==dstloc, then *w) over all the block's
    chunks at once, instead of one tensor_scalar per chunk (the DVE was the
    other dominant cost).
  - Per chunk: tensor-engine matmul aggT += V.T @ S accumulated in PSUM
    (feature-major). Block epilogue: mT = aggT + 0.1*x0T (vector), node-major
    h' = relu(mT.T @ W_hat_l) via one matmul (W_hat_l = beta_l*W_l +
    (1-beta_l)*I folded on host; 0.9 folded into edge weights) + scalar relu,
    DMA to the layer's shard buffer.
  - AllGather of the 6250-row shard between layers gives every core the full
    h (bf16) for the next layer's gathers. Final layer writes the external
    output directly (host concatenates shards).
"""

import numpy as np

N = 50000
E = 800000
D = 128
L = 8
ALPHA = 0.1
THETA = 0.5
NCORES = 8
P = 128
LO_N = 32768                # int16 index limit: rows [0,LO_N) vs [LO_N,N)
CAP = 96                    # max chunks per (group, half) gather (Q7 scratch)

_NEFF_CACHE = {}
_LAST_IN_MAPS = None
# ablation knobs (timing experiments only)
_GATHER_ON = True
_SEL_ON = True
_MM_ON = True
_AG_OFF = False
_VP_BUFS = 2
_SP_BUFS = 4


def _preprocess(edge_index, edge_weight, n, ns, nb, ncores):
    """Sort/partition edges; build per-core arrays + shared schedule.

    Chunk schedule (identical on all cores): per block b, mlo[b]/mhi[b]
    chunks (cross-core max).  dstloc/wgt columns are BLOCK-major (block b's
    lo chunks then hi chunks at kb0[b]).  The gather streams are GROUP-major:
    groups pack consecutive blocks with sum(mlo)<=CAP and sum(mhi)<=CAP; per
    group the lo stream holds all blocks' lo chunks (block order), then the
    hi stream.  idx16 columns follow the gather-stream order.
    """
    src = np.asarray(edge_index[0], dtype=np.int64)
    dst = np.asarray(edge_index[1], dtype=np.int64)
    w = np.asarray(edge_weight, dtype=np.float32) * (1.0 - ALPHA)

    order = np.argsort(dst, kind="stable")
    src_s, dst_s, w_s = src[order], dst[order], w[order]

    core = dst_s // ns
    local = dst_s - core * ns
    block = local // P
    dstloc = (local - block * P).astype(np.float32)
    hi = (src_s >= LO_N).astype(np.int64)

    # cross-core-max chunks per (block, half)
    cbh = (core * nb + block) * 2 + hi
    counts = np.bincount(cbh, minlength=ncores * nb * 2).reshape(ncores, nb, 2)
    cpg = -(-counts.max(axis=0) // P)  # [nb, 2]
    mlo = cpg[:, 0].astype(int)
    mhi = cpg[:, 1].astype(int)
    mb = mlo + mhi
    kb0 = np.concatenate([[0], np.cumsum(mb)[:-1]]).astype(int)
    nch = int(mb.sum())

    # groups of consecutive blocks
    groups = []  # (blocks, Mlo, Mhi, olo, ohi, vlo_off, vhi_off)
    cur_blocks, cur_lo, cur_hi = [], 0, 0
    for b in range(nb):
        if cur_blocks and (cur_lo + mlo[b] > CAP or cur_hi + mhi[b] > CAP):
            groups.append((cur_blocks, cur_lo, cur_hi))
            cur_blocks, cur_lo, cur_hi = [], 0, 0
        cur_blocks.append(b)
        cur_lo += mlo[b]
        cur_hi += mhi[b]
    groups.append((cur_blocks, cur_lo, cur_hi))

    ginfo = []
    off = 0
    for blocks, Mlo, Mhi in groups:
        vlo_off, vhi_off = {}, {}
        al, ah = 0, 0
        for b in blocks:
            vlo_off[b] = al
            vhi_off[b] = ah
            al += mlo[b]
            ah += mhi[b]
        ginfo.append(dict(blocks=blocks, Mlo=Mlo, Mhi=Mhi,
                          olo=off, ohi=off + Mlo,
                          vlo_off=vlo_off, vhi_off=vhi_off))
        off += Mlo + Mhi
    assert off == nch
    glo = max(g["Mlo"] for g in ginfo)
    ghi = max(g["Mhi"] for g in ginfo)
    mbmax = int(mb.max())

    import ml_dtypes
    bf16 = ml_dtypes.bfloat16

    dl_arrs, w_arrs, idx_arrs = [], [], []
    core_starts = np.searchsorted(core, np.arange(ncores + 1))
    for c in range(ncores):
        s0, s1 = core_starts[c], core_starts[c + 1]
        blk = block[s0:s1]
        bstart = np.searchsorted(blk, np.arange(nb + 1)) + s0
        da = np.zeros(nch * P, dtype=np.float32)
        wa = np.zeros(nch * P, dtype=np.float32)
        i16 = np.zeros(nch * P, dtype=np.int16)  # gather-stream order
        for g in ginfo:
            for b in g["blocks"]:
                e0, e1 = bstart[b], bstart[b + 1]
                seg_src = src_s[e0:e1]
                seg_hi = seg_src >= LO_N
                for h in range(2):
                    m = seg_hi if h else ~seg_hi
                    gsrc = seg_src[m]
                    cnt = len(gsrc)
                    # dstloc/wgt: block-major position
                    kpos = (kb0[b] + (mlo[b] if h else 0)) * P
                    da[kpos:kpos + cnt] = dstloc[e0:e1][m]
                    wa[kpos:kpos + cnt] = w_s[e0:e1][m]
                    # idx: gather-stream position
                    ipos = ((g["ohi"] + g["vhi_off"][b]) if h
                            else (g["olo"] + g["vlo_off"][b])) * P
                    i16[ipos:ipos + cnt] = \
                        (gsrc - (LO_N if h else 0)).astype(np.int16)
        dl_arrs.append(np.ascontiguousarray(
            da.reshape(nch, P).T.astype(bf16)))
        w_arrs.append(np.ascontiguousarray(
            wa.reshape(nch, P).T.astype(bf16)))
        # wrap idx streams per (group, half): [16, M*8] repeated to 8 stripes
        ia = np.zeros((16, nch * 8), dtype=np.int16)
        for g in ginfo:
            for (o, m) in ((g["olo"], g["Mlo"]), (g["ohi"], g["Mhi"])):
                if m:
                    sub = i16[o * P:(o + m) * P]
                    ia[:, o * 8:(o + m) * 8] = sub.reshape(m * 8, 16).T
        idx_arrs.append(np.ascontiguousarray(np.tile(ia, (8, 1))))

    sched = dict(ginfo=ginfo, mlo=mlo, mhi=mhi, kb0=kb0, nch=nch,
                 glo=glo, ghi=ghi, mbmax=mbmax)
    return dl_arrs, w_arrs, idx_arrs, sched


def _build(nc, *, n, ns, nb, sched, n_layers):
    import concourse.mybir as mybir
    import concourse.tile as tile
    from concourse.masks import make_identity

    f32 = mybir.dt.float32
    i32 = mybir.dt.int32
    fh = mybir.dt.bfloat16
    nch = sched["nch"]
    glo, ghi, mbmax = sched["glo"], sched["ghi"], sched["mbmax"]
    mlo, mhi, kb0 = sched["mlo"], sched["mhi"], sched["kb0"]

    x_t = nc.dram_tensor("x_shard", [ns, D], f32, kind="ExternalInput")
    wlin_t = nc.dram_tensor("w_lin", [D, D], f32, kind="ExternalInput")
    blin_t = nc.dram_tensor("b_lin", [D], f32, kind="ExternalInput")
    what_t = nc.dram_tensor("w_hat", [n_layers, D, D], f32, kind="ExternalInput")
    dstloc_t = nc.dram_tensor("dst_loc", [P, nch], fh, kind="ExternalInput")
    wgt_t = nc.dram_tensor("wgt", [P, nch], fh, kind="ExternalInput")
    idx16_t = nc.dram_tensor("idx16", [P, nch * 8], mybir.dt.int16,
                             kind="ExternalInput")
    out_t = nc.dram_tensor("h_out", [ns, D], f32, kind="ExternalOutput")

    hbuf = [nc.dram_tensor(f"h_full{i}", [n, D], fh, addr_space="Shared")
            for i in range(2)]
    shbuf = [nc.dram_tensor(f"h_shard{i}", [ns, D], fh) for i in range(2)]
    rg = [list(range(NCORES))]

    n_full = ns // P
    last = ns - n_full * P
    blk_rows = [P] * n_full + ([last] if last else [])
    assert len(blk_rows) == nb

    with tile.TileContext(nc) as tc:
        with (
            tc.tile_pool(name="res", bufs=1) as res,
            tc.tile_pool(name="xp", bufs=3) as xp,
            tc.tile_pool(name="vp", bufs=_VP_BUFS) as vp,
            tc.tile_pool(name="sp", bufs=_SP_BUFS) as sp,
            tc.tile_pool(name="mp", bufs=4) as mp,
            tc.tile_pool(name="hp", bufs=6) as hp,
            tc.tile_pool(name="ps", bufs=2, space="PSUM") as ps,
            tc.tile_pool(name="psa", bufs=2, space="PSUM") as psa,
        ):
            ident = res.tile([P, P], f32, tag="ident")
            make_identity(nc, ident[:])
            iota_i = res.tile([P, P], i32, tag="iotai")
            nc.gpsimd.iota(iota_i[:], pattern=[[1, P]], base=0,
                           channel_multiplier=0)
            iota_f = res.tile([P, P], fh, tag="iotaf")
            nc.vector.tensor_copy(out=iota_f[:], in_=iota_i[:])

            wlin_s = res.tile([P, D], f32, tag="wlin")
            nc.sync.dma_start(out=wlin_s[:], in_=wlin_t[:])
            blin_s = res.tile([P, 1], f32, tag="blin")
            nc.sync.dma_start(out=blin_s[:], in_=blin_t[:, None])
            what_s = res.tile([P, n_layers * D], f32, tag="what")
            for l in range(n_layers):
                nc.sync.dma_start(out=what_s[:, l * D:(l + 1) * D],
                                  in_=what_t[l, :, :])
            dstloc = res.tile([P, nch], fh, tag="dstloc")
            nc.sync.dma_start(out=dstloc[:], in_=dstloc_t[:])
            wgt = res.tile([P, nch], fh, tag="wgt")
            nc.sync.dma_start(out=wgt[:], in_=wgt_t[:])
            idx16 = res.tile([P, nch * 8], mybir.dt.int16, tag="idx16")
            nc.sync.dma_start(out=idx16[:], in_=idx16_t[:])

            x0sT = res.tile([P, ns], f32, tag="x0sT")  # 0.1 * relu(xW+b).T

            # ---- prologue: x0 = relu(x @ W_lin + b) ----
            for b in range(nb):
                rows = blk_rows[b]
                r0 = b * P
                xb = xp.tile([P, D], f32, tag="xb")
                nc.sync.dma_start(out=xb[:rows, :], in_=x_t[r0:r0 + rows, :])
                xbT_ps = ps.tile([P, P], f32, tag="tr", space="PSUM")
                nc.tensor.transpose(out=xbT_ps[:, :rows], in_=xb[:rows, :],
                                    identity=ident[:rows, :rows])
                xbT = xp.tile([P, P], f32, tag="xbT")
                nc.vector.tensor_copy(out=xbT[:, :rows], in_=xbT_ps[:, :rows])
                ps2 = ps.tile([P, P], f32, tag="dense", space="PSUM")
                nc.tensor.matmul(out=ps2[:, :rows], lhsT=wlin_s[:],
                                 rhs=xbT[:, :rows], start=True, stop=True)
                x0Tb = xp.tile([P, P], f32, tag="x0Tb")
                nc.scalar.activation(out=x0Tb[:, :rows], in_=ps2[:, :rows],
                                     func=mybir.ActivationFunctionType.Relu,
                                     bias=blin_s[:, :1], scale=1.0)
                nc.vector.tensor_scalar(
                    out=x0sT[:, r0:r0 + rows], in0=x0Tb[:, :rows],
                    scalar1=ALPHA, scalar2=None, op0=mybir.AluOpType.mult)
                x0_ps = ps.tile([P, P], f32, tag="tr", space="PSUM")
                nc.tensor.transpose(out=x0_ps[:rows, :], in_=x0Tb[:, :rows],
                                    identity=ident[:])
                x0b = hp.tile([P, D], fh, tag="hb")
                nc.vector.tensor_copy(out=x0b[:rows, :], in_=x0_ps[:rows, :])
                nc.sync.dma_start(out=shbuf[0][r0:r0 + rows, :],
                                  in_=x0b[:rows, :])

            if _AG_OFF:
                nc.sync.dma_start(out=hbuf[0][0:ns, :], in_=shbuf[0][:])
            else:
                nc.gpsimd.collective_compute(
                    "AllGather", mybir.AluOpType.bypass, replica_groups=rg,
                    ins=[shbuf[0][:]], outs=[hbuf[0][:]])

            # ---- layers ----
            gq = 0
            for l in range(n_layers):
                h_cur = hbuf[l % 2]
                is_last = l == n_layers - 1
                for g in sched["ginfo"]:
                    Mlo, Mhi = g["Mlo"], g["Mhi"]
                    vlo = vp.tile([P, glo, D], fh, tag="vlo")
                    vhi = vp.tile([P, max(ghi, 1), D], fh, tag="vhi")
                    if _GATHER_ON and Mlo:
                        nc.gpsimd.dma_gather(
                            vlo[:, :Mlo, :], h_cur[0:LO_N, :],
                            idx16[:, g["olo"] * 8:(g["olo"] + Mlo) * 8],
                            num_idxs=Mlo * P, num_idxs_reg=Mlo * P,
                            elem_size=D, single_packet=False,
                            queue_num=gq % 4)
                        gq += 1
                    if _GATHER_ON and Mhi:
                        nc.gpsimd.dma_gather(
                            vhi[:, :Mhi, :], h_cur[LO_N:n, :],
                            idx16[:, g["ohi"] * 8:(g["ohi"] + Mhi) * 8],
                            num_idxs=Mhi * P, num_idxs_reg=Mhi * P,
                            elem_size=D, single_packet=False,
                            queue_num=gq % 4)
                        gq += 1
                    for b in g["blocks"]:
                        rows = blk_rows[b]
                        r0 = b * P
                        mb = mlo[b] + mhi[b]
                        k0 = kb0[b]
                        sb = sp.tile([P, mbmax, P], fh, tag="s")
                        if _SEL_ON:
                            iota_bc = iota_f[:].unsqueeze(1).broadcast_to(
                                (P, mb, P))
                            dl_bc = dstloc[:, k0:k0 + mb].unsqueeze(
                                2).broadcast_to((P, mb, P))
                            w_bc = wgt[:, k0:k0 + mb].unsqueeze(
                                2).broadcast_to((P, mb, P))
                            nc.vector.tensor_tensor(
                                out=sb[:, :mb, :], in0=iota_bc, in1=dl_bc,
                                op=mybir.AluOpType.is_equal)
                            nc.vector.tensor_tensor(
                                out=sb[:, :mb, :], in0=sb[:, :mb, :],
                                in1=w_bc, op=mybir.AluOpType.mult)
                        aggT = psa.tile([P, P], f32, tag="agg", space="PSUM")
                        if _MM_ON:
                            ki = 0
                            for j in range(mlo[b]):
                                nc.tensor.matmul(
                                    out=aggT[:],
                                    lhsT=vlo[:, g["vlo_off"][b] + j, :],
                                    rhs=sb[:, ki, :],
                                    start=ki == 0, stop=ki == mb - 1)
                                ki += 1
                            for j in range(mhi[b]):
                                nc.tensor.matmul(
                                    out=aggT[:],
                                    lhsT=vhi[:, g["vhi_off"][b] + j, :],
                                    rhs=sb[:, ki, :],
                                    start=ki == 0, stop=ki == mb - 1)
                                ki += 1
                        else:
                            nc.vector.memset(aggT[:], 0.0)
                        mT = mp.tile([P, P], f32, tag="mT")
                        nc.vector.tensor_tensor(
                            out=mT[:, :rows], in0=aggT[:, :rows],
                            in1=x0sT[:, r0:r0 + rows], op=mybir.AluOpType.add)
                        ps2 = ps.tile([P, P], f32, tag="dense", space="PSUM")
                        nc.tensor.matmul(out=ps2[:rows, :], lhsT=mT[:, :rows],
                                         rhs=what_s[:, l * D:(l + 1) * D],
                                         start=True, stop=True)
                        hb = hp.tile([P, D], f32 if is_last else fh, tag="hbo")
                        nc.scalar.activation(
                            out=hb[:rows, :], in_=ps2[:rows, :],
                            func=mybir.ActivationFunctionType.Relu)
                        dst_dram = out_t if is_last else shbuf[(l + 1) % 2]
                        nc.sync.dma_start(out=dst_dram[r0:r0 + rows, :],
                                          in_=hb[:rows, :])
                if not is_last:
                    if _AG_OFF:
                        nc.sync.dma_start(out=hbuf[(l + 1) % 2][0:ns, :],
                                          in_=shbuf[(l + 1) % 2][:])
                    else:
                        nc.gpsimd.collective_compute(
                            "AllGather", mybir.AluOpType.bypass,
                            replica_groups=rg,
                            ins=[shbuf[(l + 1) % 2][:]],
                            outs=[hbuf[(l + 1) % 2][:]])
    return nc


def _run(inputs, *, n, e, n_layers, ncores=NCORES):
    import concourse.bacc as bacc
    from concourse.bass_utils import run_bass_kernel_spmd

    x = np.asarray(inputs["x"], dtype=np.float32)
    edge_weight = np.asarray(inputs["edge_weight"], dtype=np.float32)
    w_lin = np.asarray(inputs["W_lin"], dtype=np.float32)
    b_lin = np.asarray(inputs["b_lin"], dtype=np.float32)
    w_convs = np.asarray(inputs["W_convs"], dtype=np.float32)
    edge_index = np.asarray(inputs["edge_index"])

    ns = n // ncores
    nb = -(-ns // P)

    betas = np.log(THETA / np.arange(1, n_layers + 1) + 1.0).astype(np.float32)
    eye = np.eye(D, dtype=np.float32)
    w_hat = np.stack([betas[l] * w_convs[l] + (1.0 - betas[l]) * eye
                      for l in range(n_layers)]).astype(np.float32)

    dl_arrs, w_arrs, idx_arrs, sched = _preprocess(
        edge_index, edge_weight, n, ns, nb, ncores)

    key = (n, e, n_layers, sched["nch"], sched["glo"], sched["ghi"])
    if key not in _NEFF_CACHE:
        nc = bacc.Bacc("TRN2", target_bir_lowering=False, debug=False,
                       num_devices=ncores, num_swdge_queues=4)
        _build(nc, n=n, ns=ns, nb=nb, sched=sched, n_layers=n_layers)
        nc.compile()
        _NEFF_CACHE[key] = nc
    nc = _NEFF_CACHE[key]

    in_maps = []
    for c in range(ncores):
        in_maps.append({
            "x_shard": np.ascontiguousarray(x[c * ns:(c + 1) * ns]),
            "w_lin": w_lin, "b_lin": b_lin, "w_hat": w_hat,
            "dst_loc": dl_arrs[c], "wgt": w_arrs[c],
            "idx16": idx_arrs[c],
        })
    global _LAST_IN_MAPS
    _LAST_IN_MAPS = in_maps
    res = run_bass_kernel_spmd(nc, in_maps, list(range(ncores)))
    out = np.concatenate([res.results[c]["h_out"] for c in range(ncores)],
                         axis=0)
    return out


def kernel(x, edge_weight, W_lin, b_lin, W_convs, edge_index):
    return _run(
        dict(x=x, edge_weight=edge_weight, W_lin=W_lin, b_lin=b_lin,
             W_convs=W_convs, edge_index=edge_index),
        n=N, e=E, n_layers=L)


# revision 14
# speedup vs baseline: 1.0421x; 1.0421x over previous
"""GCNII (8-layer GCN2Conv stack) on 8 Trainium2 NeuronCores.

Strategy (1D graph parallel over destination nodes):
  - Core c owns destination rows [c*NS, (c+1)*NS) of every layer's output.
  - Edges partitioned by dst, sorted by dst, grouped per 128-dst-node "block"
    into 128-edge chunks (lo/hi split at LO_N so gather indices fit int16),
    padded with zero-weight edges to a cross-core-uniform chunk schedule.
  - Gathers are MERGED: consecutive blocks form groups of <=CAP chunks per
    src-half; one dma_gather per (group, half) fetches all rows (the Q7 SWDGE
    fixed cost ~7us/instruction dominated the old per-(block,half) scheme).
  - Selectors S[e, j] = (j == dstloc[e]) * w[e] are built per block with TWO
    broadcast tensor_tensor ops (iota==dstloc, then *w) over all the block's
    chunks at once, instead of one tensor_scalar per chunk (the DVE was the
    other dominant cost).
  - Per chunk: tensor-engine matmul aggT += V.T @ S accumulated in PSUM
    (feature-major). Block epilogue: mT = aggT + 0.1*x0T (vector), node-major
    h' = relu(mT.T @ W_hat_l) via one matmul (W_hat_l = beta_l*W_l +
    (1-beta_l)*I folded on host; 0.9 folded into edge weights) + scalar relu,
    DMA to the layer's shard buffer.
  - AllGather of the 6250-row shard between layers gives every core the full
    h (bf16) for the next layer's gathers. Final layer writes the external
    output directly (host concatenates shards).
"""

import numpy as np

N = 50000
E = 800000
D = 128
L = 8
ALPHA = 0.1
THETA = 0.5
NCORES = 8
P = 128
LO_N = 32768                # int16 index limit: rows [0,LO_N) vs [LO_N,N)
CAP = 48                    # max chunks per (group, half) gather

_NEFF_CACHE = {}
_LAST_IN_MAPS = None
# ablation knobs (timing experiments only)
_GATHER_ON = True
_SEL_ON = True
_MM_ON = True
_AG_OFF = False
_VP_BUFS = 2
_SP_BUFS = 4
_SINGLE_PACKET = False
_GSPLIT = 2                 # queue-gathers per (group, half) region


def _preprocess(edge_index, edge_weight, n, ns, nb, ncores):
    """Sort/partition edges; build per-core arrays + shared schedule.

    Chunk schedule (identical on all cores): per block b, mlo[b]/mhi[b]
    chunks (cross-core max).  dstloc/wgt columns are BLOCK-major (block b's
    lo chunks then hi chunks at kb0[b]).  The gather streams are GROUP-major:
    groups pack consecutive blocks with sum(mlo)<=CAP and sum(mhi)<=CAP; per
    group the lo stream holds all blocks' lo chunks (block order), then the
    hi stream.  idx16 columns follow the gather-stream order.
    """
    src = np.asarray(edge_index[0], dtype=np.int64)
    dst = np.asarray(edge_index[1], dtype=np.int64)
    w = np.asarray(edge_weight, dtype=np.float32) * (1.0 - ALPHA)

    order = np.argsort(dst, kind="stable")
    src_s, dst_s, w_s = src[order], dst[order], w[order]

    core = dst_s // ns
    local = dst_s - core * ns
    block = local // P
    dstloc = (local - block * P).astype(np.float32)
    hi = (src_s >= LO_N).astype(np.int64)

    # cross-core-max chunks per (block, half)
    cbh = (core * nb + block) * 2 + hi
    counts = np.bincount(cbh, minlength=ncores * nb * 2).reshape(ncores, nb, 2)
    cpg = -(-counts.max(axis=0) // P)  # [nb, 2]
    mlo = cpg[:, 0].astype(int)
    mhi = cpg[:, 1].astype(int)
    mb = mlo + mhi
    kb0 = np.concatenate([[0], np.cumsum(mb)[:-1]]).astype(int)
    nch = int(mb.sum())

    # groups of consecutive blocks
    groups = []  # (blocks, Mlo, Mhi, olo, ohi, vlo_off, vhi_off)
    cur_blocks, cur_lo, cur_hi = [], 0, 0
    for b in range(nb):
        if cur_blocks and (cur_lo + mlo[b] > CAP or cur_hi + mhi[b] > CAP):
            groups.append((cur_blocks, cur_lo, cur_hi))
            cur_blocks, cur_lo, cur_hi = [], 0, 0
        cur_blocks.append(b)
        cur_lo += mlo[b]
        cur_hi += mhi[b]
    groups.append((cur_blocks, cur_lo, cur_hi))

    ginfo = []
    off = 0
    for blocks, Mlo, Mhi in groups:
        vlo_off, vhi_off = {}, {}
        al, ah = 0, 0
        for b in blocks:
            vlo_off[b] = al
            vhi_off[b] = ah
            al += mlo[b]
            ah += mhi[b]
        ginfo.append(dict(blocks=blocks, Mlo=Mlo, Mhi=Mhi,
                          olo=off, ohi=off + Mlo,
                          vlo_off=vlo_off, vhi_off=vhi_off))
        off += Mlo + Mhi
    assert off == nch
    glo = max(g["Mlo"] for g in ginfo)
    ghi = max(g["Mhi"] for g in ginfo)
    mbmax = int(mb.max())

    import ml_dtypes
    bf16 = ml_dtypes.bfloat16

    s_arrs, idx_arrs = [], []
    core_starts = np.searchsorted(core, np.arange(ncores + 1))
    for c in range(ncores):
        s0, s1 = core_starts[c], core_starts[c + 1]
        blk = block[s0:s1]
        bstart = np.searchsorted(blk, np.arange(nb + 1)) + s0
        da = np.zeros(nch * P, dtype=np.int64)
        wa = np.zeros(nch * P, dtype=np.float32)
        i16 = np.zeros(nch * P, dtype=np.int16)  # gather-stream order
        for g in ginfo:
            for b in g["blocks"]:
                e0, e1 = bstart[b], bstart[b + 1]
                seg_src = src_s[e0:e1]
                seg_hi = seg_src >= LO_N
                for h in range(2):
                    m = seg_hi if h else ~seg_hi
                    gsrc = seg_src[m]
                    cnt = len(gsrc)
                    # dstloc/wgt: block-major position
                    kpos = (kb0[b] + (mlo[b] if h else 0)) * P
                    da[kpos:kpos + cnt] = dstloc[e0:e1][m].astype(np.int64)
                    wa[kpos:kpos + cnt] = w_s[e0:e1][m]
                    # idx: gather-stream position
                    ipos = ((g["ohi"] + g["vhi_off"][b]) if h
                            else (g["olo"] + g["vlo_off"][b])) * P
                    i16[ipos:ipos + cnt] = \
                        (gsrc - (LO_N if h else 0)).astype(np.int16)
        # host-built selector: S[p, k, c] = w[p, k] * (dstloc[p, k] == c)
        # (identical for every layer; streamed from HBM instead of built on
        # the vector engine)
        dl2 = da.reshape(nch, P).T            # [P, nch] lane-major
        w2 = wa.reshape(nch, P).T
        s3 = np.zeros((P, nch, P), dtype=bf16)
        np.put_along_axis(s3, dl2[:, :, None], w2[:, :, None].astype(bf16),
                          axis=2)
        s_arrs.append(np.ascontiguousarray(s3))
        # wrap idx streams per (group, half): [16, M*8] repeated to 8 stripes
        ia = np.zeros((16, nch * 8), dtype=np.int16)
        for g in ginfo:
            for (o, m) in ((g["olo"], g["Mlo"]), (g["ohi"], g["Mhi"])):
                if m:
                    sub = i16[o * P:(o + m) * P]
                    ia[:, o * 8:(o + m) * 8] = sub.reshape(m * 8, 16).T
        idx_arrs.append(np.ascontiguousarray(np.tile(ia, (8, 1))))

    # group start column (block-major) for S slicing
    for g in ginfo:
        g["kg0"] = int(kb0[g["blocks"][0]])
        g["wg"] = int(sum(mlo[b] + mhi[b] for b in g["blocks"]))
    wgmax = max(g["wg"] for g in ginfo)
    sched = dict(ginfo=ginfo, mlo=mlo, mhi=mhi, kb0=kb0, nch=nch,
                 glo=glo, ghi=ghi, mbmax=mbmax, wgmax=wgmax)
    return s_arrs, idx_arrs, sched


def _build(nc, *, n, ns, nb, sched, n_layers):
    import concourse.mybir as mybir
    import concourse.tile as tile
    from concourse.masks import make_identity

    f32 = mybir.dt.float32
    fh = mybir.dt.bfloat16
    nch = sched["nch"]
    glo, ghi = sched["glo"], sched["ghi"]
    wgmax = sched["wgmax"]
    mlo, mhi, kb0 = sched["mlo"], sched["mhi"], sched["kb0"]

    x_t = nc.dram_tensor("x_shard", [ns, D], f32, kind="ExternalInput")
    wlin_t = nc.dram_tensor("w_lin", [D, D], f32, kind="ExternalInput")
    blin_t = nc.dram_tensor("b_lin", [D], f32, kind="ExternalInput")
    what_t = nc.dram_tensor("w_hat", [n_layers, D, D], f32, kind="ExternalInput")
    sel_t = nc.dram_tensor("sel", [P, nch, P], fh, kind="ExternalInput")
    idx16_t = nc.dram_tensor("idx16", [P, nch * 8], mybir.dt.int16,
                             kind="ExternalInput")
    out_t = nc.dram_tensor("h_out", [ns, D], f32, kind="ExternalOutput")

    hbuf = [nc.dram_tensor(f"h_full{i}", [n, D], fh, addr_space="Shared")
            for i in range(2)]
    shbuf = [nc.dram_tensor(f"h_shard{i}", [ns, D], fh) for i in range(2)]
    rg = [list(range(NCORES))]

    n_full = ns // P
    last = ns - n_full * P
    blk_rows = [P] * n_full + ([last] if last else [])
    assert len(blk_rows) == nb

    with tile.TileContext(nc) as tc:
        with (
            tc.tile_pool(name="res", bufs=1) as res,
            tc.tile_pool(name="xp", bufs=3) as xp,
            tc.tile_pool(name="vp", bufs=_VP_BUFS) as vp,
            tc.tile_pool(name="sp", bufs=_SP_BUFS) as sp,
            tc.tile_pool(name="mp", bufs=4) as mp,
            tc.tile_pool(name="hp", bufs=6) as hp,
            tc.tile_pool(name="ps", bufs=2, space="PSUM") as ps,
            tc.tile_pool(name="psa", bufs=2, space="PSUM") as psa,
        ):
            ident = res.tile([P, P], f32, tag="ident")
            make_identity(nc, ident[:])

            wlin_s = res.tile([P, D], f32, tag="wlin")
            nc.sync.dma_start(out=wlin_s[:], in_=wlin_t[:])
            blin_s = res.tile([P, 1], f32, tag="blin")
            nc.sync.dma_start(out=blin_s[:], in_=blin_t[:, None])
            what_s = res.tile([P, n_layers * D], f32, tag="what")
            for l in range(n_layers):
                nc.sync.dma_start(out=what_s[:, l * D:(l + 1) * D],
                                  in_=what_t[l, :, :])
            idx16 = res.tile([P, nch * 8], mybir.dt.int16, tag="idx16")
            nc.sync.dma_start(out=idx16[:], in_=idx16_t[:])

            x0sT = res.tile([P, ns], f32, tag="x0sT")  # 0.1 * relu(xW+b).T

            # ---- prologue: x0 = relu(x @ W_lin + b) ----
            for b in range(nb):
                rows = blk_rows[b]
                r0 = b * P
                xb = xp.tile([P, D], f32, tag="xb")
                nc.sync.dma_start(out=xb[:rows, :], in_=x_t[r0:r0 + rows, :])
                xbT_ps = ps.tile([P, P], f32, tag="tr", space="PSUM")
                nc.tensor.transpose(out=xbT_ps[:, :rows], in_=xb[:rows, :],
                                    identity=ident[:rows, :rows])
                xbT = xp.tile([P, P], f32, tag="xbT")
                nc.vector.tensor_copy(out=xbT[:, :rows], in_=xbT_ps[:, :rows])
                ps2 = ps.tile([P, P], f32, tag="dense", space="PSUM")
                nc.tensor.matmul(out=ps2[:, :rows], lhsT=wlin_s[:],
                                 rhs=xbT[:, :rows], start=True, stop=True)
                x0Tb = xp.tile([P, P], f32, tag="x0Tb")
                nc.scalar.activation(out=x0Tb[:, :rows], in_=ps2[:, :rows],
                                     func=mybir.ActivationFunctionType.Relu,
                                     bias=blin_s[:, :1], scale=1.0)
                nc.vector.tensor_scalar(
                    out=x0sT[:, r0:r0 + rows], in0=x0Tb[:, :rows],
                    scalar1=ALPHA, scalar2=None, op0=mybir.AluOpType.mult)
                x0_ps = ps.tile([P, P], f32, tag="tr", space="PSUM")
                nc.tensor.transpose(out=x0_ps[:rows, :], in_=x0Tb[:, :rows],
                                    identity=ident[:])
                x0b = hp.tile([P, D], fh, tag="hb")
                nc.vector.tensor_copy(out=x0b[:rows, :], in_=x0_ps[:rows, :])
                nc.sync.dma_start(out=shbuf[0][r0:r0 + rows, :],
                                  in_=x0b[:rows, :])

            if _AG_OFF:
                nc.sync.dma_start(out=hbuf[0][0:ns, :], in_=shbuf[0][:])
            else:
                nc.gpsimd.collective_compute(
                    "AllGather", mybir.AluOpType.bypass, replica_groups=rg,
                    ins=[shbuf[0][:]], outs=[hbuf[0][:]])

            # ---- layers ----
            gq = 0
            for l in range(n_layers):
                h_cur = hbuf[l % 2]
                is_last = l == n_layers - 1
                for g in sched["ginfo"]:
                    Mlo, Mhi = g["Mlo"], g["Mhi"]
                    vlo = vp.tile([P, glo, D], fh, tag="vlo")
                    vhi = vp.tile([P, max(ghi, 1), D], fh, tag="vhi")
                    kg0, wg = g["kg0"], g["wg"]
                    sb = sp.tile([P, wgmax, P], fh, tag="s")
                    if _SEL_ON:
                        nc.sync.dma_start(out=sb[:, :wg, :],
                                          in_=sel_t[:, kg0:kg0 + wg, :])
                    for (vt, M, o, src_ap) in (
                            (vlo, Mlo, g["olo"], h_cur[0:LO_N, :]),
                            (vhi, Mhi, g["ohi"], h_cur[LO_N:n, :])):
                        if not (_GATHER_ON and M):
                            continue
                        step = -(-M // _GSPLIT)
                        for c0 in range(0, M, step):
                            m = min(step, M - c0)
                            nc.gpsimd.dma_gather(
                                vt[:, c0:c0 + m, :], src_ap,
                                idx16[:, (o + c0) * 8:(o + c0 + m) * 8],
                                num_idxs=m * P, num_idxs_reg=m * P,
                                elem_size=D, single_packet=_SINGLE_PACKET,
                                queue_num=gq % 4)
                            gq += 1
                    for b in g["blocks"]:
                        rows = blk_rows[b]
                        r0 = b * P
                        mb = mlo[b] + mhi[b]
                        ks = kb0[b] - kg0
                        aggT = psa.tile([P, P], f32, tag="agg", space="PSUM")
                        if _MM_ON:
                            ki = 0
                            for j in range(mlo[b]):
                                nc.tensor.matmul(
                                    out=aggT[:],
                                    lhsT=vlo[:, g["vlo_off"][b] + j, :],
                                    rhs=sb[:, ks + ki, :],
                                    start=ki == 0, stop=ki == mb - 1)
                                ki += 1
                            for j in range(mhi[b]):
                                nc.tensor.matmul(
                                    out=aggT[:],
                                    lhsT=vhi[:, g["vhi_off"][b] + j, :],
                                    rhs=sb[:, ks + ki, :],
                                    start=ki == 0, stop=ki == mb - 1)
                                ki += 1
                        else:
                            nc.vector.memset(aggT[:], 0.0)
                        mT = mp.tile([P, P], f32, tag="mT")
                        nc.vector.tensor_tensor(
                            out=mT[:, :rows], in0=aggT[:, :rows],
                            in1=x0sT[:, r0:r0 + rows], op=mybir.AluOpType.add)
                        ps2 = ps.tile([P, P], f32, tag="dense", space="PSUM")
                        nc.tensor.matmul(out=ps2[:rows, :], lhsT=mT[:, :rows],
                                         rhs=what_s[:, l * D:(l + 1) * D],
                                         start=True, stop=True)
                        hb = hp.tile([P, D], f32 if is_last else fh, tag="hbo")
                        nc.scalar.activation(
                            out=hb[:rows, :], in_=ps2[:rows, :],
                            func=mybir.ActivationFunctionType.Relu)
                        dst_dram = out_t if is_last else shbuf[(l + 1) % 2]
                        nc.sync.dma_start(out=dst_dram[r0:r0 + rows, :],
                                          in_=hb[:rows, :])
                if not is_last:
                    if _AG_OFF:
                        nc.sync.dma_start(out=hbuf[(l + 1) % 2][0:ns, :],
                                          in_=shbuf[(l + 1) % 2][:])
                    else:
                        nc.gpsimd.collective_compute(
                            "AllGather", mybir.AluOpType.bypass,
                            replica_groups=rg,
                            ins=[shbuf[(l + 1) % 2][:]],
                            outs=[hbuf[(l + 1) % 2][:]])
    return nc


def _run(inputs, *, n, e, n_layers, ncores=NCORES):
    import concourse.bacc as bacc
    from concourse.bass_utils import run_bass_kernel_spmd

    x = np.asarray(inputs["x"], dtype=np.float32)
    edge_weight = np.asarray(inputs["edge_weight"], dtype=np.float32)
    w_lin = np.asarray(inputs["W_lin"], dtype=np.float32)
    b_lin = np.asarray(inputs["b_lin"], dtype=np.float32)
    w_convs = np.asarray(inputs["W_convs"], dtype=np.float32)
    edge_index = np.asarray(inputs["edge_index"])

    ns = n // ncores
    nb = -(-ns // P)

    betas = np.log(THETA / np.arange(1, n_layers + 1) + 1.0).astype(np.float32)
    eye = np.eye(D, dtype=np.float32)
    w_hat = np.stack([betas[l] * w_convs[l] + (1.0 - betas[l]) * eye
                      for l in range(n_layers)]).astype(np.float32)

    s_arrs, idx_arrs, sched = _preprocess(
        edge_index, edge_weight, n, ns, nb, ncores)

    key = (n, e, n_layers, sched["nch"], sched["glo"], sched["ghi"])
    if key not in _NEFF_CACHE:
        nc = bacc.Bacc("TRN2", target_bir_lowering=False, debug=False,
                       num_devices=ncores, num_swdge_queues=4)
        _build(nc, n=n, ns=ns, nb=nb, sched=sched, n_layers=n_layers)
        nc.compile()
        _NEFF_CACHE[key] = nc
    nc = _NEFF_CACHE[key]

    in_maps = []
    for c in range(ncores):
        in_maps.append({
            "x_shard": np.ascontiguousarray(x[c * ns:(c + 1) * ns]),
            "w_lin": w_lin, "b_lin": b_lin, "w_hat": w_hat,
            "sel": s_arrs[c], "idx16": idx_arrs[c],
        })
    global _LAST_IN_MAPS
    _LAST_IN_MAPS = in_maps
    res = run_bass_kernel_spmd(nc, in_maps, list(range(ncores)))
    out = np.concatenate([res.results[c]["h_out"] for c in range(ncores)],
                         axis=0)
    return out


def kernel(x, edge_weight, W_lin, b_lin, W_convs, edge_index):
    return _run(
        dict(x=x, edge_weight=edge_weight, W_lin=W_lin, b_lin=b_lin,
             W_convs=W_convs, edge_index=edge_index),
        n=N, e=E, n_layers=L)
